# revision 10
# baseline (speedup 1.0000x reference)
"""Correlation cost-volume kernel for Trainium2 (Bass/Tile).

Problem: in1, in2: [B=8, C=128, H=96, W=128] fp32.
Output: [B, 81, H, W] where out[b, dy*9+dx, y, x] =
    mean_c( in1[b,c,y,x] * in2_pad[b,c,y+dy,x+dx] ),
with in2 zero-padded by 4 in both spatial dims (max_displacement=4).

Strategy (data-parallel over batch, one sample per NeuronCore):
  - For each in1 row y, compute the Gram band against the 9 surrounding
    (padded) in2 rows with TensorE matmuls: stationary = in1[:, y, :]
    ([C=128, W=128]), moving = padded in2 rows y..y+8 ([C=128, 3x136] per
    matmul, 3 matmuls) -> PSUM G[x, (dy, x')] where
    G = sum_c in1[c,y,x] * in2p[c, y+dy, x'].
  - Copy PSUM->SBUF in 32-partition groups, keeping only the 40-wide
    window W[x, dy, u] = G[x, dy, 32*(x//32)+u] each pixel group needs
    (pure access patterns only: mixed partition+byte strides in DMA APs
    miscompute on HW - the DGE wraps the per-partition byte carry).
  - Extract the banded taps with 32 partition-strided SBUF->SBUF DMAs
    (s = x mod 32): t2[x, dy*9+dx] = W[x, dy, s+dx].
  - PE-transpose the [128 x, 81 k] band tile to [81, 128] and DMA
    straight into the output cost volume rows, scaling by 1/C on the
    way.

Matmuls run in float32r (full PE rate for N>=256, ~7e-4 scale-relative
error vs fp64 reference -- measured on hardware).
"""

import numpy as np

import concourse.bass as bass
import concourse.mybir as mybir
from concourse import bacc
from concourse.bass_utils import run_bass_kernel_spmd
from concourse.masks import make_identity
from concourse.tile import TileContext

B = 8
C = 128
H = 96
W = 128
D = 9  # 2*max_disp + 1
K = D * D  # 81 output channels
PAD = 4
WP = W + 2 * PAD  # 136
FP32 = mybir.dt.float32
FP32R = mybir.dt.float32r

N_CORES = 8


def build_bass(h: int = H):
    """Build the per-core Bass program for a [C, h, W] sample."""
    hp = h + 2 * PAD
    nc = bacc.Bacc(None, target_bir_lowering=False)
    in1 = nc.dram_tensor("in1", [C, h, W], FP32R, kind="ExternalInput")
    # in2p is host-padded: [C, h+8, W+8] with zeros in the 4-wide borders.
    in2p = nc.dram_tensor("in2p", [C, hp, WP], FP32R, kind="ExternalInput")
    out = nc.dram_tensor("out", [K, h, W], FP32, kind="ExternalOutput")
    out_t = out[:, :, :].tensor

    with TileContext(nc) as tc:
        with (
            tc.tile_pool(name="big", bufs=1) as big_pool,
            tc.tile_pool(name="work", bufs=3) as work_pool,
            tc.tile_pool(name="gpsum", bufs=2, space="PSUM") as gpsum,
            tc.tile_pool(name="tpsum", bufs=2, space="PSUM") as tpsum,
        ):
            s1 = big_pool.tile([C, h, W], FP32R, name="s1")
            s2p = big_pool.tile([C, hp, WP], FP32R, name="s2p")
            ident = big_pool.tile([128, 128], FP32, name="ident")
            make_identity(nc, ident)

            # Load inputs in row-chunks so compute can start early.
            nchunk = 4
            rows1 = (h + nchunk - 1) // nchunk
            for i in range(0, h, rows1):
                r = min(rows1, h - i)
                nc.sync.dma_start(s1[:, i : i + r, :], in1[:, i : i + r, :])
            rows2 = (hp + nchunk - 1) // nchunk
            for i in range(0, hp, rows2):
                r = min(rows2, hp - i)
                nc.sync.dma_start(s2p[:, i : i + r, :], in2p[:, i : i + r, :])

            for y in range(h):
                # --- 3 matmuls: G[x, (dy, x')] over dy triplets ---
                gp = gpsum.tile([128, 3, 512], FP32, name="gp", tag="gp")
                for j in range(3):
                    nc.tensor.matmul(
                        gp[:, j, 0 : 3 * WP],
                        s1[:, y, :],
                        s2p[:, y + 3 * j : y + 3 * j + 3, :],
                        start=True,
                        stop=True,
                    )

                # --- PSUM -> SBUF windowed copy (per 32-partition group) ---
                # W[x, dy, u] = G[x, dy, n = 32*(x//32) + u], u in [0, 40).
                # The group base 32g is absorbed into each copy's offsets, so
                # every AP is pure (no partition/byte mixed strides); engine
                # partition bases must be multiples of 32.
                wt = work_pool.tile([128, D, 40], FP32, name="wt", tag="wt")
                # view gp as [p, j, r, n] with n = moving col within dy row
                gp_r = gp[:, :, 0 : 3 * WP].rearrange(
                    "p j (r n) -> p j r n", r=3
                )
                wt_r = wt[:, :, :].rearrange("p (j r) u -> p j r u", j=3)
                for g in range(4):
                    src = gp_r[32 * g : 32 * g + 32, :, :, 32 * g : 32 * g + 40]
                    dst = wt_r[32 * g : 32 * g + 32, :, :, :]
                    if g % 2 == 0:
                        nc.scalar.activation(
                            dst, src, mybir.ActivationFunctionType.Copy
                        )
                    else:
                        nc.vector.tensor_copy(dst, src)

                # --- band extraction: 32 partition-strided SBUF->SBUF DMAs ---
                # For s = x mod 32: t2[x, dy*9+dx] = W[x, dy, s+dx]
                t2 = work_pool.tile([128, K], FP32, name="t2", tag="t2")
                for s in range(32):
                    src = wt[s::32, :, s : s + D]
                    dst = t2[s::32, :]
                    eng = nc.scalar if s % 2 == 0 else nc.sync
                    eng.dma_start(dst, src)

                # --- PE transpose [128, 81] -> [81, 128] ---
                tt = tpsum.tile([K, 128], FP32, name="tt", tag="tt")
                nc.tensor.transpose(tt[:, :], t2[:, :], ident[:, :])

                # --- scale by 1/C and copy to SBUF ---
                to = work_pool.tile([K, 128], FP32, name="to", tag="to")
                nc.scalar.activation(
                    to[:, :],
                    tt[:, :],
                    mybir.ActivationFunctionType.Copy,
                    scale=1.0 / C,
                )

                # --- store: partition k = dy*9+dx -> out[k, y, :] ---
                nc.sync.dma_start(out[:, y, :], to[:, :])

    nc.compile()
    return nc


_cached = {}


def _get_nc(h: int):
    if h not in _cached:
        _cached[h] = build_bass(h)
    return _cached[h]


def _pad_in2(in2: np.ndarray) -> np.ndarray:
    # [C, h, W] -> [C, h+8, W+8] zero-padded, contiguous fp32
    return np.pad(
        in2, ((0, 0), (PAD, PAD), (PAD, PAD)), mode="constant"
    ).astype(np.float32, copy=False)


def kernel(**inputs: np.ndarray) -> np.ndarray:
    in1 = np.ascontiguousarray(inputs["in1"], dtype=np.float32)
    in2 = np.ascontiguousarray(inputs["in2"], dtype=np.float32)
    assert in1.shape == (B, C, H, W), in1.shape

    nc = _get_nc(H)
    in_maps = [
        {
            "in1": np.ascontiguousarray(in1[b]),
            "in2p": np.ascontiguousarray(_pad_in2(in2[b])),
        }
        for b in range(B)
    ]
    res = run_bass_kernel_spmd(nc, in_maps, core_ids=list(range(N_CORES)))
    return np.stack([r["out"] for r in res.results], axis=0)
